# revision 40
# baseline (speedup 1.0000x reference)
"""Trainium2 Bass kernel for nn_ActorCritic (GIN message passing actor-critic).

Strategy (data-parallel over graphs, per sharding hint):
- 8 graphs x 225 nodes; core c owns graph c (225 nodes).
- segment_sum over the 14400 random (cross-graph) edges is done as a dense
  bf16 matmul with a host-built adjacency shard AT_c[src, dst_local].
- GIN MLP runs feature-major ([feat, node]) so BatchNorm reductions are along
  the free dim. BN statistics are global over all 1800 nodes: each core
  computes its 225 pre-BN columns, an AllGather (bf16) shares them, every core
  redundantly computes the tiny global stats locally. rsqrt for BN runs on the
  vector engine (fast-inverse-sqrt + Newton) so ScalarE only ever loads the
  Relu and Exp activation tables.
- Actor head factorized: logits[i,j] = w2 . relu(s + Q[:,i] + P[:,j]);
  hidden tiles pack 4 i's x 32 k on 128 partitions (bf16); logits accumulate
  into one PSUM tile via shifted block-diagonal weights. Softmax is local.
"""

import sys

if "/opt/trn_rl_repo" not in sys.path:
    sys.path.insert(0, "/opt/trn_rl_repo")

import ml_dtypes
import numpy as np

import concourse.bacc as bacc
import concourse.bass as bass
import concourse.mybir as mybir
import concourse.tile as tile
from concourse.bass_isa import ReduceOp
from concourse.bass_utils import run_bass_kernel_spmd

P = 128
N = 225            # nodes per graph (= per core)
B = 8              # graphs = cores
NA = N * B         # 1800 total nodes
K = 15             # src tiles of 128
NAP = K * P        # 1920 padded
HID = 64
HA = 32            # actor hidden
HC = 32            # critic hidden
BN_EPS = 1e-5
F32 = mybir.dt.float32
BF16 = mybir.dt.bfloat16
NPBF16 = ml_dtypes.bfloat16

# wpack column layout ([128, WCOLS])
C_W1A0 = 0     # rows 0:2   g0_w1 (b1 cancels under BN)   cols 0:64
C_W2A0 = 64    # rows 0:65  [g0_w2; g0_b2]                cols 64:128
C_W1A1 = 128   # rows 0:64  g1_w1                          cols 128:192
C_W2A1 = 192   # rows 0:65  [g1_w2; g1_b2]                cols 192:256
C_A1S = 256    # rows 0:65  [a_w1[0:64]; a_b1]             cols 256:288
C_A1P = 288    # rows 0:64  a_w1[64:128]                   cols 288:320
C_A1Q = 320    # rows 0:64  a_w1[128:192]                  cols 320:352
C_W24 = 352    # rows 0:128 block-diag a_w2                cols 352:356
C_C1 = 356     # rows 0:65  [c_w1; c_b1]                   cols 356:388
C_C2 = 388     # rows 0:33  [c_w2; c_b2]                   cols 388:389
C_G0 = 389     # rows 0:64  g0_gamma
C_B0 = 390     # rows 0:64  g0_beta
C_G1 = 391     # rows 0:64  g1_gamma
C_B1 = 392     # rows 0:64  g1_beta
WCOLS = 400

import os
ADJ_BF16 = os.environ.get("KADJ", "1") == "1"     # adjacency matmul path
DENSE_BF16 = os.environ.get("KDENSE", "1") == "1" # dense-layer matmul path
ACTOR_BF16 = os.environ.get("KACTOR", "1") == "1" # actor hid/w24 matmuls
V1_STYLE = os.environ.get("KV1", "0") == "1"      # unused master switch
COPY_NEW = os.environ.get("KCOPY", "1") == "1"    # vector copy for h1fmcs
EMBA_NEW = os.environ.get("KEMBA", "1") == "1"    # reduce emb from PSUM featsp
SP4_NEW = os.environ.get("KSP4", "1") == "1"      # all-DVE sp4 build
SPLIT_NEW = os.environ.get("KSPLIT", "1") == "1"  # ACT_EVERY actor relu split
ADT = BF16 if ADJ_BF16 else F32
DDT = BF16 if DENSE_BF16 else F32
XDT = BF16 if ACTOR_BF16 else F32

NBLK = 32          # actor loop count (block b covers i-blocks b and b+32)
NIB = 57           # i blocks of 4 (225 valid i -> 57 blocks)
ACT_EVERY = 3      # 1 in ACT_EVERY actor relus go to ScalarE, rest to DVE


def _bn_finish(nc, sb, s1, s2, ncols, gamma, beta, name):
    """Global BN (scale, shift) [HID,1] f32 from raw sum / sum-of-squares."""
    ncols = float(ncols)
    mean = sb.tile([HID, 1], F32, name=f"mean_{name}")
    var = sb.tile([HID, 1], F32, name=f"var_{name}")
    nc.vector.tensor_scalar_mul(mean[:], s1[:], 1.0 / ncols)
    nc.vector.tensor_scalar_mul(s2[:], s2[:], 1.0 / ncols)
    nc.vector.tensor_tensor(out=var[:], in0=mean[:], in1=mean[:], op=mybir.AluOpType.mult)
    nc.vector.tensor_tensor(out=var[:], in0=s2[:], in1=var[:], op=mybir.AluOpType.subtract)
    nc.vector.tensor_scalar_add(var[:], var[:], BN_EPS)
    nc.scalar.activation(out=var[:], in_=var[:], func=mybir.ActivationFunctionType.Sqrt)
    inv = sb.tile([HID, 1], F32, name=f"inv_{name}")
    nc.vector.reciprocal(inv[:], var[:])
    scale = sb.tile([HID, 1], F32, name=f"scale_{name}")
    shift = sb.tile([HID, 1], F32, name=f"shift_{name}")
    nc.vector.tensor_tensor(out=scale[:], in0=gamma, in1=inv[:], op=mybir.AluOpType.mult)
    nc.vector.tensor_tensor(out=inv[:], in0=mean[:], in1=scale[:], op=mybir.AluOpType.mult)
    nc.vector.tensor_tensor(out=shift[:], in0=beta, in1=inv[:], op=mybir.AluOpType.subtract)
    return scale, shift


def build_program():
    nc = bacc.Bacc("TRN2", target_bir_lowering=False, debug=False, num_devices=B)

    at_d = nc.dram_tensor("at_c", [NAP, N], ADT, kind="ExternalInput")
    atf_d = nc.dram_tensor("at_full", [NAP, NA], ADT, kind="ExternalInput")
    x_d = nc.dram_tensor("x_all", [NAP, 2], ADT, kind="ExternalInput")
    xt_d = nc.dram_tensor("xt_c", [2, N], F32, kind="ExternalInput")
    xtf_d = nc.dram_tensor("xt_all", [2, NA], F32, kind="ExternalInput")
    wp_d = nc.dram_tensor("wpack", [P, WCOLS], DDT, kind="ExternalInput")
    pi_d = nc.dram_tensor("pi", [N, N], F32, kind="ExternalOutput")
    val_d = nc.dram_tensor("val", [1, 1], F32, kind="ExternalOutput")

    rg = [list(range(B))]
    RELU = mybir.ActivationFunctionType.Relu
    EXP = mybir.ActivationFunctionType.Exp

    with tile.TileContext(nc) as tc:
        with (
            tc.tile_pool(name="sb", bufs=1) as sb,
            tc.tile_pool(name="ps", bufs=3, space="PSUM") as ps,
            tc.tile_pool(name="psb", bufs=1, space="PSUM") as psb,
            tc.tile_pool(name="psl", bufs=1, space="PSUM") as psl,
            tc.tile_pool(name="hidp", bufs=6) as hidp,
            tc.tile_pool(name="dram", bufs=1, space="DRAM") as dram,
        ):
            # Warm-up collective: absorbs the one-time collective-entry
            # setup (~25us observed on the first collective) during the
            # input-DMA phase instead of on the critical path.
            # Content is irrelevant (output never read), so no input DMA:
            # the collective has zero dependencies and issues immediately.
            warm_in = dram.tile([1, 16], F32, name="warm_in")
            warm_out = dram.tile([B, 16], F32, name="warm_out", addr_space="Shared")
            nc.gpsimd.collective_compute(
                "AllGather", mybir.AluOpType.bypass, replica_groups=rg,
                ins=[warm_in[:]], outs=[warm_out[:]],
            )

            # ---------------- loads ----------------
            at_sb = sb.tile([P, K, N], ADT, name="at_sb")
            at_v = at_d[:, :].rearrange("(k p) j -> p k j", p=P)
            for g, (k0, k1) in enumerate([(0, 4), (4, 8), (8, 12), (12, 15)]):
                eng = nc.sync if g % 2 == 0 else nc.scalar
                eng.dma_start(at_sb[:, k0:k1, :], at_v[:, k0:k1, :])
            atf_sb = sb.tile([P, K, NA], ADT, name="atf_sb")
            for k in range(K):
                eng = nc.sync if k % 2 == 0 else nc.scalar
                eng.dma_start(atf_sb[:, k, :], atf_d[k * P:(k + 1) * P, :])
            xtf_sb = sb.tile([2, NA], F32, name="xtf_sb")
            nc.sync.dma_start(xtf_sb[:], xtf_d[:, :])
            x_sb = sb.tile([P, K, 2], ADT, name="x_sb")
            nc.sync.dma_start(x_sb[:], x_d[:, :].rearrange("(k p) c -> p k c", p=P))
            xt_sb = sb.tile([2, N], F32, name="xt_sb")
            nc.sync.dma_start(xt_sb[:], xt_d[:, :])
            wp = sb.tile([P, WCOLS], DDT, name="wp")
            nc.scalar.dma_start(wp[:], wp_d[:, :])

            gamma0 = wp[0:HID, C_G0:C_G0 + 1]
            beta0 = wp[0:HID, C_B0:C_B0 + 1]
            gamma1 = wp[0:HID, C_G1:C_G1 + 1]
            beta1 = wp[0:HID, C_B1:C_B1 + 1]

            # ---------------- GIN layer 0 ----------------
            agg0 = ps.tile([2, N], F32, name="agg0", tag="bank")
            for k in range(K):
                nc.tensor.matmul(agg0[:], x_sb[:, k, :], at_sb[:, k, :],
                                 start=(k == 0), stop=(k == K - 1))
            hin0 = sb.tile([2, N], DDT, name="hin0")
            nc.vector.tensor_tensor(out=hin0[:], in0=agg0[:], in1=xt_sb[:],
                                    op=mybir.AluOpType.add)
            pre0c = ps.tile([HID, N], F32, name="pre0c", tag="bank")
            nc.tensor.matmul(pre0c[:], wp[0:2, C_W1A0:C_W1A0 + HID], hin0[:],
                             start=True, stop=True)

            # L0 for ALL nodes locally (replicated) — runs under the ~80us
            # first-collective bootstrap, so no gather is needed for BN0.
            NCH = 4
            CW = NA // NCH   # 450
            agg0a = psb.tile([2, NCH, 512], F32, name="agg0a", tag="big")
            for c in range(NCH):
                for k in range(K):
                    nc.tensor.matmul(agg0a[0:2, c, 0:CW], x_sb[:, k, :],
                                     atf_sb[:, k, c * CW:(c + 1) * CW],
                                     start=(k == 0), stop=(k == K - 1))
            hin0a = sb.tile([2, NCH, 512], DDT, name="hin0a")
            for c in range(NCH):
                nc.vector.tensor_tensor(out=hin0a[0:2, c, 0:CW],
                                        in0=agg0a[0:2, c, 0:CW],
                                        in1=xtf_sb[:, c * CW:(c + 1) * CW],
                                        op=mybir.AluOpType.add)
            pre0a = psb.tile([HID, NCH, 512], F32, name="pre0a", tag="big")
            for c in range(NCH):
                nc.tensor.matmul(pre0a[:, c, 0:CW], wp[0:2, C_W1A0:C_W1A0 + HID],
                                 hin0a[0:2, c, 0:CW], start=True, stop=True)
            # contiguous f32 staging of all pre-BN values (avoids strided
            # PSUM reads for the stats/relu passes)
            pre0s = sb.tile([HID, NA], F32, name="pre0s")
            for c in range(NCH):
                eng = nc.vector if c % 2 == 0 else nc.scalar
                if c % 2 == 0:
                    eng.tensor_copy(pre0s[:, c * CW:(c + 1) * CW],
                                    pre0a[:, c, 0:CW])
                else:
                    eng.copy(pre0s[:, c * CW:(c + 1) * CW], pre0a[:, c, 0:CW])

            junk = sb.tile([HID, NA], F32, name="junk")
            s1g = sb.tile([HID, 1], F32, name="s1g")
            s2g = sb.tile([HID, 1], F32, name="s2g")
            nc.vector.reduce_sum(out=s1g[:], in_=pre0s[:],
                                 axis=mybir.AxisListType.X)
            nc.scalar.activation(out=junk[:], in_=pre0s[:],
                                 func=mybir.ActivationFunctionType.Square,
                                 accum_out=s2g[:])
            scale0, shift0 = _bn_finish(nc, sb, s1g, s2g, NA, gamma0, beta0, "l0")

            # act0 (all nodes, feature-major bf16, +ones row); own-node copy
            act0 = sb.tile([HID + 1, NAP], DDT, name="act0")
            nc.vector.memset(act0[HID:HID + 1, :], 1.0)
            nc.vector.memset(act0[0:HID, NA:NAP], 0.0)
            nc.scalar.activation(out=act0[0:HID, 0:NA], in_=pre0s[:],
                                 func=RELU, bias=shift0[:], scale=scale0[:])
            act0c = sb.tile([HID + 1, N], DDT, name="act0c")
            nc.vector.memset(act0c[HID:HID + 1, :], 1.0)
            nc.scalar.activation(out=act0c[0:HID, :], in_=pre0c[:],
                                 func=RELU, bias=shift0[:], scale=scale0[:])

            # h1 all nodes node-major (bf16) and own nodes feature-major
            h1nm = sb.tile([P, K, HID], ADT, name="h1nm")
            for k in range(K):
                hp = ps.tile([P, HID], F32, name=f"hp{k}", tag="bank")
                nc.tensor.matmul(hp[:], act0[:, k * P:(k + 1) * P],
                                 wp[0:HID + 1, C_W2A0:C_W2A0 + HID],
                                 start=True, stop=True)
                if k % 2 == 0:
                    nc.vector.tensor_copy(h1nm[:, k, :], hp[:])
                else:
                    nc.scalar.copy(h1nm[:, k, :], hp[:])
            h1fmc = ps.tile([HID, N], F32, name="h1fmc", tag="bank")
            nc.tensor.matmul(h1fmc[:], wp[0:HID + 1, C_W2A0:C_W2A0 + HID], act0c[:],
                             start=True, stop=True)
            h1fmcs = sb.tile([HID, N], F32, name="h1fmcs")
            if not COPY_NEW:
                nc.scalar.copy(h1fmcs[:], h1fmc[:])
            else:
                nc.vector.tensor_copy(h1fmcs[:], h1fmc[:])

            # ---------------- GIN layer 1 ----------------
            agg1 = ps.tile([HID, N], F32, name="agg1", tag="bank")
            for k in range(K):
                nc.tensor.matmul(agg1[:], h1nm[:, k, :], at_sb[:, k, :],
                                 start=(k == 0), stop=(k == K - 1))
            hin1 = sb.tile([HID, N], DDT, name="hin1")
            nc.vector.tensor_tensor(out=hin1[:], in0=agg1[:], in1=h1fmcs[:],
                                    op=mybir.AluOpType.add)
            pre1c = ps.tile([HID, N], F32, name="pre1c", tag="bank")
            nc.tensor.matmul(pre1c[:], wp[0:HID, C_W1A1:C_W1A1 + HID], hin1[:],
                             start=True, stop=True)

            # only the BN1 statistics cross cores: [64, 2] f32 per rank
            sloc = sb.tile([HID, 2], F32, name="sloc")
            nc.vector.reduce_sum(out=sloc[:, 0:1], in_=pre1c[:],
                                 axis=mybir.AxisListType.X)
            nc.scalar.activation(out=junk[:, 0:N], in_=pre1c[:],
                                 func=mybir.ActivationFunctionType.Square,
                                 accum_out=sloc[:, 1:2])
            ag_in1 = dram.tile([HID, 2], F32, name="ag_in1")
            ag_out1 = dram.tile([HID * B, 2], F32, name="ag_out1", addr_space="Shared")
            nc.sync.dma_start(ag_in1[:], sloc[:])
            nc.gpsimd.collective_compute(
                "AllGather", mybir.AluOpType.bypass, replica_groups=rg,
                ins=[ag_in1[:]], outs=[ag_out1[:]],
            )
            stat1 = sb.tile([HID, B, 2], F32, name="stat1")
            nc.sync.dma_start(stat1[:], ag_out1[:].rearrange("(r f) j -> f r j", f=HID))
            s1h = sb.tile([HID, 1], F32, name="s1h")
            s2h = sb.tile([HID, 1], F32, name="s2h")
            nc.vector.reduce_sum(out=s1h[:], in_=stat1[:, :, 0:1],
                                 axis=mybir.AxisListType.XY)
            nc.vector.reduce_sum(out=s2h[:], in_=stat1[:, :, 1:2],
                                 axis=mybir.AxisListType.XY)
            scale1, shift1 = _bn_finish(nc, sb, s1h, s2h, NA, gamma1, beta1, "l1")

            act1c = sb.tile([HID + 1, N], DDT, name="act1c")
            nc.vector.memset(act1c[HID:HID + 1, :], 1.0)
            nc.scalar.activation(out=act1c[0:HID, :], in_=pre1c[:],
                                 func=RELU, bias=shift1[:], scale=scale1[:])

            featsp = ps.tile([HID, N], F32, name="featsp", tag="bank")
            nc.tensor.matmul(featsp[:], wp[0:HID + 1, C_W2A1:C_W2A1 + HID], act1c[:],
                             start=True, stop=True)
            feats = sb.tile([HID, N], DDT, name="feats")
            nc.vector.tensor_copy(feats[:], featsp[:])

            # ---------------- pooling + critic ----------------
            emba = sb.tile([HID + 1, 1], DDT, name="emba")
            nc.vector.memset(emba[HID:HID + 1, :], 1.0)
            embr = sb.tile([HID, 1], F32, name="embr")
            nc.vector.reduce_sum(out=embr[:], in_=featsp[:] if EMBA_NEW else feats[:],
                                 axis=mybir.AxisListType.X)
            nc.vector.tensor_scalar_mul(emba[0:HID, :], embr[:], 1.0 / N)

            v1p = ps.tile([HC, 1], F32, name="v1p", tag="bank")
            nc.tensor.matmul(v1p[:], wp[0:HID + 1, C_C1:C_C1 + HC], emba[:],
                             start=True, stop=True)
            v1a = sb.tile([HC + 1, 1], DDT, name="v1a")
            nc.vector.memset(v1a[HC:HC + 1, :], 1.0)
            nc.scalar.activation(out=v1a[0:HC, :], in_=v1p[:], func=RELU)
            valp = ps.tile([1, 1], F32, name="valp", tag="bank")
            nc.tensor.matmul(valp[:], wp[0:HC + 1, C_C2:C_C2 + 1], v1a[:],
                             start=True, stop=True)
            vals = sb.tile([1, 1], F32, name="vals")
            nc.vector.tensor_copy(vals[:], valp[:])
            nc.sync.dma_start(val_d[:, :], vals[:])

            # ---------------- actor head ----------------
            pp = ps.tile([HA, N], F32, name="pp", tag="bank")
            nc.tensor.matmul(pp[:], wp[0:HID, C_A1P:C_A1P + HA], feats[:],
                             start=True, stop=True)
            qp = ps.tile([HA, N], F32, name="qp", tag="bank")
            nc.tensor.matmul(qp[:], wp[0:HID, C_A1Q:C_A1Q + HA], feats[:],
                             start=True, stop=True)
            spp = ps.tile([HA, 1], F32, name="spp", tag="bank")
            nc.tensor.matmul(spp[:], wp[0:HID + 1, C_A1S:C_A1S + HA], emba[:],
                             start=True, stop=True)
            ssb = sb.tile([HA, 1], F32, name="ssb")
            nc.vector.tensor_copy(ssb[:], spp[:])

            # SP4 = 4x partition-stacked (P + s)
            sp4 = sb.tile([P, N], XDT, name="sp4")
            for ii in range(4):
                if (not SP4_NEW) and ii % 2 == 1:
                    nc.scalar.activation(out=sp4[HA * ii:HA * (ii + 1), :], in_=pp[:],
                                         func=mybir.ActivationFunctionType.Identity,
                                         bias=ssb[:])
                else:
                    nc.vector.tensor_scalar_add(sp4[HA * ii:HA * (ii + 1), :],
                                                pp[:], ssb[:])

            # Q4[32*ii + k, b] = Q[k, 4b + ii]
            qsb = sb.tile([HA, 228], F32, name="qsb")
            nc.vector.memset(qsb[:, N:228], 0.0)
            nc.vector.tensor_copy(qsb[:, 0:N], qp[:])
            q4 = sb.tile([P, 2 * NBLK], F32, name="q4")
            nc.vector.memset(q4[:], 0.0)
            qsb_r = qsb[:].rearrange("k (b i) -> k b i", i=4)
            for ii in range(4):
                nc.vector.tensor_copy(q4[HA * ii:HA * (ii + 1), 0:NIB],
                                      qsb_r[:, :, ii])

            # block-diagonal w2 stacks: lhsT_b is [128, 32], placed at column
            # 4*(b%8)+ii; matmul b accumulates into logits[32g:32g+32] with
            # tile_position=(0, 32g), g=b//8 — four independent PE col-groups
            # run concurrently.
            w24dev = sb.tile([P, NBLK, HA], XDT, name="w24dev")
            nc.gpsimd.memset(w24dev[:], 0.0)
            for b in range(NBLK):
                off = 4 * (b % 8)
                nc.vector.tensor_copy(w24dev[:, b, off:off + 4],
                                      wp[0:P, C_W24:C_W24 + 4])

            def relu_hid(eng_i, out_ap, bias_ap):
                if eng_i % 4 == 0:
                    nc.scalar.activation(out=out_ap, in_=sp4[:], func=RELU,
                                         bias=bias_ap)
                else:
                    nc.vector.tensor_scalar(
                        out=out_ap, in0=sp4[:], scalar1=bias_ap, scalar2=0.0,
                        op0=mybir.AluOpType.add, op1=mybir.AluOpType.max)

            # free size padded to 512 so each partition slice is bank-aligned
            logits_t = psl.tile([P, 512], F32, name="logits_t")
            # interleave groups so consecutive matmuls hit different col-groups
            order = [8 * (i % 4) + (i // 4) for i in range(NBLK)]
            for n_i, b in enumerate(order):
                g = b // 8
                # [128, 512] with region 2 at col 256: both regions start
                # 512B-aligned so the DVE 4x bf16 mode engages (143ns vs
                # 445ns measured for the unaligned 225-offset variant).
                hid = hidp.tile([P, 512], XDT, name="hid", tag="hid")
                relu_hid(2 * n_i, hid[:, 0:N], q4[:, b:b + 1])
                relu_hid(2 * n_i + 1, hid[:, 256:256 + N],
                         q4[:, NBLK + b:NBLK + b + 1])
                rhs = hid[:].rearrange("p (g c) -> p g c", c=256)[:, :, 0:N]
                nc.tensor.matmul(logits_t[32 * g:32 * g + 32, 0:2 * N],
                                 w24dev[:, b, :], rhs,
                                 start=(b % 8 == 0), stop=(b % 8 == 7),
                                 tile_position=(0, 32 * g),
                                 skip_group_check=True)

            # ---------------- softmax over all 50625 ----------------
            # Region 1 (cols 0:225): i=0..127 on all 128 partitions; region 2
            # (cols 225:450): i=128..224 on partitions 0:97. Partitions 97:128
            # of region 2 are garbage (i>=225): never read.
            NR2 = N - P
            rowmax = sb.tile([P, 1], F32, name="rowmax")
            rm2 = sb.tile([P, 1], F32, name="rm2")
            nc.vector.reduce_max(out=rowmax[:], in_=logits_t[:, 0:N],
                                 axis=mybir.AxisListType.X)
            nc.vector.reduce_max(out=rm2[0:NR2, :], in_=logits_t[0:NR2, N:2 * N],
                                 axis=mybir.AxisListType.X)
            nc.vector.tensor_tensor(out=rowmax[0:NR2, :], in0=rowmax[0:NR2, :],
                                    in1=rm2[0:NR2, :], op=mybir.AluOpType.max)
            gmax = sb.tile([P, 1], F32, name="gmax")
            nc.gpsimd.partition_all_reduce(gmax[:], rowmax[:], P, ReduceOp.max)
            nc.vector.tensor_scalar_mul(gmax[:], gmax[:], -1.0)

            esb = sb.tile([P, 2 * N], F32, name="esb")
            rowsum = sb.tile([P, 1], F32, name="rowsum")
            rs2 = sb.tile([P, 1], F32, name="rs2")
            nc.scalar.activation(out=esb[:, 0:N], in_=logits_t[:, 0:N],
                                 func=EXP, bias=gmax[:], accum_out=rowsum[:])
            nc.scalar.activation(out=esb[0:NR2, N:2 * N], in_=logits_t[0:NR2, N:2 * N],
                                 func=EXP, bias=gmax[0:NR2, :],
                                 accum_out=rs2[0:NR2, :])
            nc.vector.tensor_tensor(out=rowsum[0:NR2, :], in0=rowsum[0:NR2, :],
                                    in1=rs2[0:NR2, :], op=mybir.AluOpType.add)
            gsum = sb.tile([P, 1], F32, name="gsum")
            nc.gpsimd.partition_all_reduce(gsum[:], rowsum[:], P, ReduceOp.add)
            rinv = sb.tile([P, 1], F32, name="rinv")
            nc.vector.reciprocal(rinv[:], gsum[:])

            pi_sb = sb.tile([P, 2 * N], F32, name="pi_sb")
            nc.vector.tensor_scalar_mul(pi_sb[:, 0:N], esb[:, 0:N], rinv[:])
            nc.vector.tensor_scalar_mul(pi_sb[0:NR2, N:2 * N],
                                        esb[0:NR2, N:2 * N], rinv[0:NR2, :])

            nc.sync.dma_start(pi_d[0:P, :], pi_sb[0:P, 0:N])
            nc.sync.dma_start(pi_d[P:N, :], pi_sb[0:NR2, N:2 * N])

    nc.compile()
    return nc


_prog = None


def _get_program():
    global _prog
    if _prog is None:
        _prog = build_program()
    return _prog


def _build_wpack(i):
    w = np.zeros((P, WCOLS), np.float32)

    def put(col, arr):
        arr = np.asarray(arr, np.float32)
        if arr.ndim == 1:
            arr = arr[:, None]
        w[0:arr.shape[0], col:col + arr.shape[1]] = arr

    put(C_W1A0, i["g0_w1"])   # b1 cancels under BatchNorm
    put(C_W2A0, np.vstack([i["g0_w2"], i["g0_b2"][None, :]]))
    put(C_W1A1, i["g1_w1"])   # b1 cancels under BatchNorm
    put(C_W2A1, np.vstack([i["g1_w2"], i["g1_b2"][None, :]]))
    put(C_A1S, np.vstack([i["a_w1"][0:HID], i["a_b1"][None, :]]))
    put(C_A1P, i["a_w1"][HID:2 * HID])
    put(C_A1Q, i["a_w1"][2 * HID:3 * HID])
    w24 = np.zeros((P, 4), np.float32)
    for ii in range(4):
        w24[HA * ii:HA * (ii + 1), ii] = i["a_w2"][:, 0]
    put(C_W24, w24)
    put(C_C1, np.vstack([i["c_w1"], i["c_b1"][None, :]]))
    put(C_C2, np.vstack([i["c_w2"], i["c_b2"][None, :]]))
    put(C_G0, i["g0_gamma"])
    put(C_B0, i["g0_beta"])
    put(C_G1, i["g1_gamma"])
    put(C_B1, i["g1_beta"])
    return w.astype(NPBF16 if DENSE_BF16 else np.float32)


def kernel(**inputs):
    inp = {k: np.asarray(v) for k, v in inputs.items()}
    x = inp["x"].astype(np.float32)                      # [1800, 2]
    ei = inp["edge_index"].astype(np.int64)              # [2, 14400]

    adt = NPBF16 if ADJ_BF16 else np.float32
    at = np.zeros((NAP, NA), np.float32)
    np.add.at(at, (ei[0], ei[1]), 1.0)
    at = at.astype(adt)

    xp = np.zeros((NAP, 2), np.float32)
    xp[0:NA] = x
    xp = xp.astype(adt)
    xt_all = np.ascontiguousarray(x.T)
    wpack = _build_wpack(inp)

    in_maps = []
    for c in range(B):
        in_maps.append({
            "at_c": np.ascontiguousarray(at[:, c * N:(c + 1) * N]),
            "at_full": at,
            "x_all": xp,
            "xt_c": np.ascontiguousarray(x[c * N:(c + 1) * N].T),
            "xt_all": xt_all,
            "wpack": wpack,
        })

    nc = _get_program()
    res = run_bass_kernel_spmd(nc, in_maps, core_ids=list(range(B)))
    kernel._last_results = res

    pi = np.stack([res.results[c]["pi"].reshape(-1) for c in range(B)])
    val = np.concatenate([res.results[c]["val"].reshape(1, 1) for c in range(B)])
    return pi, val


# revision 42
# speedup vs baseline: 1.0176x; 1.0176x over previous
"""Trainium2 Bass kernel for nn_ActorCritic (GIN message passing actor-critic).

Strategy (data-parallel over graphs, per sharding hint):
- 8 graphs x 225 nodes; core c owns graph c (225 nodes).
- segment_sum over the 14400 random (cross-graph) edges is done as a dense
  bf16 matmul with a host-built adjacency shard AT_c[src, dst_local].
- GIN MLP runs feature-major ([feat, node]) so BatchNorm reductions are along
  the free dim. BN statistics are global over all 1800 nodes: each core
  computes its 225 pre-BN columns, an AllGather (bf16) shares them, every core
  redundantly computes the tiny global stats locally. rsqrt for BN runs on the
  vector engine (fast-inverse-sqrt + Newton) so ScalarE only ever loads the
  Relu and Exp activation tables.
- Actor head factorized: logits[i,j] = w2 . relu(s + Q[:,i] + P[:,j]);
  hidden tiles pack 4 i's x 32 k on 128 partitions (bf16); logits accumulate
  into one PSUM tile via shifted block-diagonal weights. Softmax is local.
"""

import sys

if "/opt/trn_rl_repo" not in sys.path:
    sys.path.insert(0, "/opt/trn_rl_repo")

import ml_dtypes
import numpy as np

import concourse.bacc as bacc
import concourse.bass as bass
import concourse.mybir as mybir
import concourse.tile as tile
from concourse.bass_isa import ReduceOp
from concourse.bass_utils import run_bass_kernel_spmd

P = 128
N = 225            # nodes per graph (= per core)
B = 8              # graphs = cores
NA = N * B         # 1800 total nodes
K = 15             # src tiles of 128
NAP = K * P        # 1920 padded
HID = 64
HA = 32            # actor hidden
HC = 32            # critic hidden
BN_EPS = 1e-5
F32 = mybir.dt.float32
BF16 = mybir.dt.bfloat16
NPBF16 = ml_dtypes.bfloat16

# wpack column layout ([128, WCOLS])
C_W1A0 = 0     # rows 0:2   g0_w1 (b1 cancels under BN)   cols 0:64
C_W2A0 = 64    # rows 0:65  [g0_w2; g0_b2]                cols 64:128
C_W1A1 = 128   # rows 0:64  g1_w1                          cols 128:192
C_W2A1 = 192   # rows 0:65  [g1_w2; g1_b2]                cols 192:256
C_A1S = 256    # rows 0:65  [a_w1[0:64]; a_b1]             cols 256:288
C_A1P = 288    # rows 0:64  a_w1[64:128]                   cols 288:320
C_A1Q = 320    # rows 0:64  a_w1[128:192]                  cols 320:352
C_W24 = 352    # rows 0:128 block-diag a_w2                cols 352:356
C_C1 = 356     # rows 0:65  [c_w1; c_b1]                   cols 356:388
C_C2 = 388     # rows 0:33  [c_w2; c_b2]                   cols 388:389
C_G0 = 389     # rows 0:64  g0_gamma
C_B0 = 390     # rows 0:64  g0_beta
C_G1 = 391     # rows 0:64  g1_gamma
C_B1 = 392     # rows 0:64  g1_beta
WCOLS = 400

import os
ADJ_BF16 = os.environ.get("KADJ", "1") == "1"     # adjacency matmul path
DENSE_BF16 = os.environ.get("KDENSE", "1") == "1" # dense-layer matmul path
ACTOR_BF16 = os.environ.get("KACTOR", "1") == "1" # actor hid/w24 matmuls
V1_STYLE = os.environ.get("KV1", "0") == "1"      # unused master switch
COPY_NEW = os.environ.get("KCOPY", "1") == "1"    # vector copy for h1fmcs
EMBA_NEW = os.environ.get("KEMBA", "1") == "1"    # reduce emb from PSUM featsp
SP4_NEW = os.environ.get("KSP4", "1") == "1"      # all-DVE sp4 build
SPLIT_NEW = os.environ.get("KSPLIT", "1") == "1"  # ACT_EVERY actor relu split
ADT = BF16 if ADJ_BF16 else F32
DDT = BF16 if DENSE_BF16 else F32
XDT = BF16 if ACTOR_BF16 else F32

NBLK = 32          # actor loop count (block b covers i-blocks b and b+32)
NIB = 57           # i blocks of 4 (225 valid i -> 57 blocks)
ACT_EVERY = 3      # 1 in ACT_EVERY actor relus go to ScalarE, rest to DVE


def _bn_scale_shift(nc, sb, pre_all, ncols, junk, gamma, beta, name):
    """Global BN (scale, shift) [HID,1] f32.

    pre_all is [HID, B, N+2] — per-rank blocks with local sum at col N and
    local sum-of-squares at col N+1 (computed pre-AllGather), so the global
    stats are tiny 8-element reductions instead of 1800-col passes."""
    s1 = sb.tile([HID, 1], F32, name=f"s1_{name}")
    s2 = sb.tile([HID, 1], F32, name=f"s2_{name}")
    nc.vector.reduce_sum(out=s1[:], in_=pre_all[:, :, N:N + 1],
                         axis=mybir.AxisListType.XY)
    nc.vector.reduce_sum(out=s2[:], in_=pre_all[:, :, N + 1:N + 2],
                         axis=mybir.AxisListType.XY)
    return _bn_finish(nc, sb, s1, s2, ncols, gamma, beta, name)


def _bn_finish(nc, sb, s1, s2, ncols, gamma, beta, name):
    """Global BN (scale, shift) [HID,1] f32 from raw sum / sum-of-squares."""
    ncols = float(ncols)
    mean = sb.tile([HID, 1], F32, name=f"mean_{name}")
    var = sb.tile([HID, 1], F32, name=f"var_{name}")
    nc.vector.tensor_scalar_mul(mean[:], s1[:], 1.0 / ncols)
    nc.vector.tensor_scalar_mul(s2[:], s2[:], 1.0 / ncols)
    nc.vector.tensor_tensor(out=var[:], in0=mean[:], in1=mean[:], op=mybir.AluOpType.mult)
    nc.vector.tensor_tensor(out=var[:], in0=s2[:], in1=var[:], op=mybir.AluOpType.subtract)
    nc.vector.tensor_scalar_add(var[:], var[:], BN_EPS)
    nc.scalar.activation(out=var[:], in_=var[:], func=mybir.ActivationFunctionType.Sqrt)
    inv = sb.tile([HID, 1], F32, name=f"inv_{name}")
    nc.vector.reciprocal(inv[:], var[:])
    scale = sb.tile([HID, 1], F32, name=f"scale_{name}")
    shift = sb.tile([HID, 1], F32, name=f"shift_{name}")
    nc.vector.tensor_tensor(out=scale[:], in0=gamma, in1=inv[:], op=mybir.AluOpType.mult)
    nc.vector.tensor_tensor(out=inv[:], in0=mean[:], in1=scale[:], op=mybir.AluOpType.mult)
    nc.vector.tensor_tensor(out=shift[:], in0=beta, in1=inv[:], op=mybir.AluOpType.subtract)
    return scale, shift


def build_program():
    nc = bacc.Bacc("TRN2", target_bir_lowering=False, debug=False, num_devices=B)

    at_d = nc.dram_tensor("at_c", [NAP, N], ADT, kind="ExternalInput")
    x_d = nc.dram_tensor("x_all", [NAP, 2], ADT, kind="ExternalInput")
    xt_d = nc.dram_tensor("xt_c", [2, N], F32, kind="ExternalInput")
    wp_d = nc.dram_tensor("wpack", [P, WCOLS], DDT, kind="ExternalInput")
    pi_d = nc.dram_tensor("pi", [N, N], F32, kind="ExternalOutput")
    val_d = nc.dram_tensor("val", [1, 1], F32, kind="ExternalOutput")

    rg = [list(range(B))]
    RELU = mybir.ActivationFunctionType.Relu
    EXP = mybir.ActivationFunctionType.Exp

    with tile.TileContext(nc) as tc:
        with (
            tc.tile_pool(name="sb", bufs=1) as sb,
            tc.tile_pool(name="ps", bufs=4, space="PSUM") as ps,
            tc.tile_pool(name="psl", bufs=1, space="PSUM") as psl,
            tc.tile_pool(name="hidp", bufs=6) as hidp,
            tc.tile_pool(name="dram", bufs=1, space="DRAM") as dram,
        ):
            # Warm-up collective: absorbs the one-time collective-entry
            # setup (~25us observed on the first collective) during the
            # input-DMA phase instead of on the critical path.
            # Content is irrelevant (output never read), so no input DMA:
            # the collective has zero dependencies and issues immediately.
            warm_in = dram.tile([1, 16], F32, name="warm_in")
            warm_out = dram.tile([B, 16], F32, name="warm_out", addr_space="Shared")
            nc.gpsimd.collective_compute(
                "AllGather", mybir.AluOpType.bypass, replica_groups=rg,
                ins=[warm_in[:]], outs=[warm_out[:]],
            )

            # ---------------- loads ----------------
            at_sb = sb.tile([P, K, N], ADT, name="at_sb")
            at_v = at_d[:, :].rearrange("(k p) j -> p k j", p=P)
            for g, (k0, k1) in enumerate([(0, 2), (2, 4), (4, 6), (6, 8), (8, 10),
                                          (10, 12), (12, 14), (14, 15)]):
                eng = nc.sync if g % 2 == 0 else nc.scalar
                eng.dma_start(at_sb[:, k0:k1, :], at_v[:, k0:k1, :])
            x_sb = sb.tile([P, K, 2], ADT, name="x_sb")
            nc.sync.dma_start(x_sb[:], x_d[:, :].rearrange("(k p) c -> p k c", p=P))
            xt_sb = sb.tile([2, N], F32, name="xt_sb")
            nc.sync.dma_start(xt_sb[:], xt_d[:, :])
            wp = sb.tile([P, WCOLS], DDT, name="wp")
            nc.scalar.dma_start(wp[:], wp_d[:, :])

            gamma0 = wp[0:HID, C_G0:C_G0 + 1]
            beta0 = wp[0:HID, C_B0:C_B0 + 1]
            gamma1 = wp[0:HID, C_G1:C_G1 + 1]
            beta1 = wp[0:HID, C_B1:C_B1 + 1]

            # ---------------- GIN layer 0 ----------------
            agg0 = ps.tile([2, N], F32, name="agg0", tag="bank")
            for k in range(K):
                nc.tensor.matmul(agg0[:], x_sb[:, k, :], at_sb[:, k, :],
                                 start=(k == 0), stop=(k == K - 1))
            hin0 = sb.tile([2, N], DDT, name="hin0")
            nc.vector.tensor_tensor(out=hin0[:], in0=agg0[:], in1=xt_sb[:],
                                    op=mybir.AluOpType.add)
            pre0c = ps.tile([HID, N], F32, name="pre0c", tag="bank")
            nc.tensor.matmul(pre0c[:], wp[0:2, C_W1A0:C_W1A0 + HID], hin0[:],
                             start=True, stop=True)

            # AllGather pre-BN columns + local stats: payload [64, 227]
            NS = N + 2
            ag_in0 = dram.tile([HID, NS], BF16, name="ag_in0")
            ag_out0 = dram.tile([HID * B, NS], BF16, name="ag_out0", addr_space="Shared")
            pre0cs = sb.tile([HID, NS], BF16, name="pre0cs")
            junk = sb.tile([HID, N], F32, name="junk")
            sloc = sb.tile([HID, 2], F32, name="sloc")
            nc.vector.tensor_copy(pre0cs[:, 0:N], pre0c[:])
            nc.vector.reduce_sum(out=sloc[:, 0:1], in_=pre0c[:],
                                 axis=mybir.AxisListType.X)
            nc.scalar.activation(out=junk[:], in_=pre0c[:],
                                 func=mybir.ActivationFunctionType.Square,
                                 accum_out=sloc[:, 1:2])
            nc.vector.tensor_copy(pre0cs[:, N:N + 2], sloc[:])
            nc.sync.dma_start(ag_in0[:], pre0cs[:])
            nc.gpsimd.collective_compute(
                "AllGather", mybir.AluOpType.bypass, replica_groups=rg,
                ins=[ag_in0[:]], outs=[ag_out0[:]],
            )
            pre0all = sb.tile([HID, B, NS], BF16, name="pre0all")
            ag0v = ag_out0[:].rearrange("(r f) j -> f r j", f=HID)
            for r in range(0, B, 2):
                eng = nc.sync if (r // 2) % 2 == 0 else nc.scalar
                eng.dma_start(pre0all[:, r:r + 2, :], ag0v[:, r:r + 2, :])
            scale0, shift0 = _bn_scale_shift(nc, sb, pre0all, NA, junk,
                                             gamma0, beta0, "l0")

            # act0 (all nodes, feature-major bf16, +ones row); own-node copy
            act0 = sb.tile([HID + 1, NAP], DDT, name="act0")
            nc.vector.memset(act0[HID:HID + 1, :], 1.0)
            nc.vector.memset(act0[0:HID, NA:NAP], 0.0)
            nc.scalar.activation(
                out=act0[0:HID, 0:NA].rearrange("f (r j) -> f r j", r=B),
                in_=pre0all[:, :, 0:N],
                func=RELU, bias=shift0[:], scale=scale0[:])
            act0c = sb.tile([HID + 1, N], DDT, name="act0c")
            nc.vector.memset(act0c[HID:HID + 1, :], 1.0)
            nc.scalar.activation(out=act0c[0:HID, :], in_=pre0c[:],
                                 func=RELU, bias=shift0[:], scale=scale0[:])

            # h1 all nodes node-major (bf16) and own nodes feature-major
            h1nm = sb.tile([P, K, HID], ADT, name="h1nm")
            for k in range(K):
                hp = ps.tile([P, HID], F32, name=f"hp{k}", tag="bank")
                nc.tensor.matmul(hp[:], act0[:, k * P:(k + 1) * P],
                                 wp[0:HID + 1, C_W2A0:C_W2A0 + HID],
                                 start=True, stop=True)
                if k % 2 == 0:
                    nc.vector.tensor_copy(h1nm[:, k, :], hp[:])
                else:
                    nc.scalar.copy(h1nm[:, k, :], hp[:])
            h1fmc = ps.tile([HID, N], F32, name="h1fmc", tag="bank")
            nc.tensor.matmul(h1fmc[:], wp[0:HID + 1, C_W2A0:C_W2A0 + HID], act0c[:],
                             start=True, stop=True)
            h1fmcs = sb.tile([HID, N], F32, name="h1fmcs")
            if not COPY_NEW:
                nc.scalar.copy(h1fmcs[:], h1fmc[:])
            else:
                nc.vector.tensor_copy(h1fmcs[:], h1fmc[:])

            # ---------------- GIN layer 1 ----------------
            agg1 = ps.tile([HID, N], F32, name="agg1", tag="bank")
            for k in range(K):
                nc.tensor.matmul(agg1[:], h1nm[:, k, :], at_sb[:, k, :],
                                 start=(k == 0), stop=(k == K - 1))
            hin1 = sb.tile([HID, N], DDT, name="hin1")
            nc.vector.tensor_tensor(out=hin1[:], in0=agg1[:], in1=h1fmcs[:],
                                    op=mybir.AluOpType.add)
            pre1c = ps.tile([HID, N], F32, name="pre1c", tag="bank")
            nc.tensor.matmul(pre1c[:], wp[0:HID, C_W1A1:C_W1A1 + HID], hin1[:],
                             start=True, stop=True)

            # only the BN1 statistics cross cores: [64, 2] f32 per rank
            nc.vector.reduce_sum(out=sloc[:, 0:1], in_=pre1c[:],
                                 axis=mybir.AxisListType.X)
            nc.scalar.activation(out=junk[:], in_=pre1c[:],
                                 func=mybir.ActivationFunctionType.Square,
                                 accum_out=sloc[:, 1:2])
            ag_in1 = dram.tile([HID, 2], F32, name="ag_in1")
            ag_out1 = dram.tile([HID * B, 2], F32, name="ag_out1", addr_space="Shared")
            nc.sync.dma_start(ag_in1[:], sloc[:])
            nc.gpsimd.collective_compute(
                "AllGather", mybir.AluOpType.bypass, replica_groups=rg,
                ins=[ag_in1[:]], outs=[ag_out1[:]],
            )
            stat1 = sb.tile([HID, B, 2], F32, name="stat1")
            nc.sync.dma_start(stat1[:], ag_out1[:].rearrange("(r f) j -> f r j", f=HID))
            s1h = sb.tile([HID, 1], F32, name="s1h")
            s2h = sb.tile([HID, 1], F32, name="s2h")
            nc.vector.reduce_sum(out=s1h[:], in_=stat1[:, :, 0:1],
                                 axis=mybir.AxisListType.XY)
            nc.vector.reduce_sum(out=s2h[:], in_=stat1[:, :, 1:2],
                                 axis=mybir.AxisListType.XY)
            scale1, shift1 = _bn_finish(nc, sb, s1h, s2h, NA, gamma1, beta1, "l1")

            act1c = sb.tile([HID + 1, N], DDT, name="act1c")
            nc.vector.memset(act1c[HID:HID + 1, :], 1.0)
            nc.scalar.activation(out=act1c[0:HID, :], in_=pre1c[:],
                                 func=RELU, bias=shift1[:], scale=scale1[:])

            featsp = ps.tile([HID, N], F32, name="featsp", tag="bank")
            nc.tensor.matmul(featsp[:], wp[0:HID + 1, C_W2A1:C_W2A1 + HID], act1c[:],
                             start=True, stop=True)
            feats = sb.tile([HID, N], DDT, name="feats")
            nc.vector.tensor_copy(feats[:], featsp[:])

            # ---------------- pooling + critic ----------------
            emba = sb.tile([HID + 1, 1], DDT, name="emba")
            nc.vector.memset(emba[HID:HID + 1, :], 1.0)
            embr = sb.tile([HID, 1], F32, name="embr")
            nc.vector.reduce_sum(out=embr[:], in_=featsp[:] if EMBA_NEW else feats[:],
                                 axis=mybir.AxisListType.X)
            nc.vector.tensor_scalar_mul(emba[0:HID, :], embr[:], 1.0 / N)

            v1p = ps.tile([HC, 1], F32, name="v1p", tag="bank")
            nc.tensor.matmul(v1p[:], wp[0:HID + 1, C_C1:C_C1 + HC], emba[:],
                             start=True, stop=True)
            v1a = sb.tile([HC + 1, 1], DDT, name="v1a")
            nc.vector.memset(v1a[HC:HC + 1, :], 1.0)
            nc.scalar.activation(out=v1a[0:HC, :], in_=v1p[:], func=RELU)
            valp = ps.tile([1, 1], F32, name="valp", tag="bank")
            nc.tensor.matmul(valp[:], wp[0:HC + 1, C_C2:C_C2 + 1], v1a[:],
                             start=True, stop=True)
            vals = sb.tile([1, 1], F32, name="vals")
            nc.vector.tensor_copy(vals[:], valp[:])
            nc.sync.dma_start(val_d[:, :], vals[:])

            # ---------------- actor head ----------------
            pp = ps.tile([HA, N], F32, name="pp", tag="bank")
            nc.tensor.matmul(pp[:], wp[0:HID, C_A1P:C_A1P + HA], feats[:],
                             start=True, stop=True)
            qp = ps.tile([HA, N], F32, name="qp", tag="bank")
            nc.tensor.matmul(qp[:], wp[0:HID, C_A1Q:C_A1Q + HA], feats[:],
                             start=True, stop=True)
            spp = ps.tile([HA, 1], F32, name="spp", tag="bank")
            nc.tensor.matmul(spp[:], wp[0:HID + 1, C_A1S:C_A1S + HA], emba[:],
                             start=True, stop=True)
            ssb = sb.tile([HA, 1], F32, name="ssb")
            nc.vector.tensor_copy(ssb[:], spp[:])

            # SP4 = 4x partition-stacked (P + s)
            sp4 = sb.tile([P, N], XDT, name="sp4")
            for ii in range(4):
                if (not SP4_NEW) and ii % 2 == 1:
                    nc.scalar.activation(out=sp4[HA * ii:HA * (ii + 1), :], in_=pp[:],
                                         func=mybir.ActivationFunctionType.Identity,
                                         bias=ssb[:])
                else:
                    nc.vector.tensor_scalar_add(sp4[HA * ii:HA * (ii + 1), :],
                                                pp[:], ssb[:])

            # Q4[32*ii + k, b] = Q[k, 4b + ii]
            qsb = sb.tile([HA, 228], F32, name="qsb")
            nc.vector.memset(qsb[:, N:228], 0.0)
            nc.vector.tensor_copy(qsb[:, 0:N], qp[:])
            q4 = sb.tile([P, 2 * NBLK], F32, name="q4")
            nc.vector.memset(q4[:], 0.0)
            qsb_r = qsb[:].rearrange("k (b i) -> k b i", i=4)
            for ii in range(4):
                nc.vector.tensor_copy(q4[HA * ii:HA * (ii + 1), 0:NIB],
                                      qsb_r[:, :, ii])

            # block-diagonal w2 stacks: lhsT_b is [128, 32], placed at column
            # 4*(b%8)+ii; matmul b accumulates into logits[32g:32g+32] with
            # tile_position=(0, 32g), g=b//8 — four independent PE col-groups
            # run concurrently.
            w24dev = sb.tile([P, NBLK, HA], XDT, name="w24dev")
            nc.gpsimd.memset(w24dev[:], 0.0)
            for b in range(NBLK):
                off = 4 * (b % 8)
                nc.vector.tensor_copy(w24dev[:, b, off:off + 4],
                                      wp[0:P, C_W24:C_W24 + 4])

            def relu_hid(eng_i, out_ap, bias_ap):
                if eng_i % 4 == 0:
                    nc.scalar.activation(out=out_ap, in_=sp4[:], func=RELU,
                                         bias=bias_ap)
                else:
                    nc.vector.tensor_scalar(
                        out=out_ap, in0=sp4[:], scalar1=bias_ap, scalar2=0.0,
                        op0=mybir.AluOpType.add, op1=mybir.AluOpType.max)

            # free size padded to 512 so each partition slice is bank-aligned
            logits_t = psl.tile([P, 512], F32, name="logits_t")
            # interleave groups so consecutive matmuls hit different col-groups
            order = [8 * (i % 4) + (i // 4) for i in range(NBLK)]
            for n_i, b in enumerate(order):
                g = b // 8
                # [128, 512] with region 2 at col 256: both regions start
                # 512B-aligned so the DVE 4x bf16 mode engages (143ns vs
                # 445ns measured for the unaligned 225-offset variant).
                hid = hidp.tile([P, 512], XDT, name="hid", tag="hid")
                relu_hid(2 * n_i, hid[:, 0:N], q4[:, b:b + 1])
                relu_hid(2 * n_i + 1, hid[:, 256:256 + N],
                         q4[:, NBLK + b:NBLK + b + 1])
                rhs = hid[:].rearrange("p (g c) -> p g c", c=256)[:, :, 0:N]
                nc.tensor.matmul(logits_t[32 * g:32 * g + 32, 0:2 * N],
                                 w24dev[:, b, :], rhs,
                                 start=(b % 8 == 0), stop=(b % 8 == 7),
                                 tile_position=(0, 32 * g),
                                 skip_group_check=True)

            # ---------------- softmax over all 50625 ----------------
            # Region 1 (cols 0:225): i=0..127 on all 128 partitions; region 2
            # (cols 225:450): i=128..224 on partitions 0:97. Partitions 97:128
            # of region 2 are garbage (i>=225): never read.
            NR2 = N - P
            rowmax = sb.tile([P, 1], F32, name="rowmax")
            rm2 = sb.tile([P, 1], F32, name="rm2")
            nc.vector.reduce_max(out=rowmax[:], in_=logits_t[:, 0:N],
                                 axis=mybir.AxisListType.X)
            nc.vector.reduce_max(out=rm2[0:NR2, :], in_=logits_t[0:NR2, N:2 * N],
                                 axis=mybir.AxisListType.X)
            nc.vector.tensor_tensor(out=rowmax[0:NR2, :], in0=rowmax[0:NR2, :],
                                    in1=rm2[0:NR2, :], op=mybir.AluOpType.max)
            gmax = sb.tile([P, 1], F32, name="gmax")
            nc.gpsimd.partition_all_reduce(gmax[:], rowmax[:], P, ReduceOp.max)
            nc.vector.tensor_scalar_mul(gmax[:], gmax[:], -1.0)

            esb = sb.tile([P, 2 * N], F32, name="esb")
            rowsum = sb.tile([P, 1], F32, name="rowsum")
            rs2 = sb.tile([P, 1], F32, name="rs2")
            nc.scalar.activation(out=esb[:, 0:N], in_=logits_t[:, 0:N],
                                 func=EXP, bias=gmax[:], accum_out=rowsum[:])
            nc.scalar.activation(out=esb[0:NR2, N:2 * N], in_=logits_t[0:NR2, N:2 * N],
                                 func=EXP, bias=gmax[0:NR2, :],
                                 accum_out=rs2[0:NR2, :])
            nc.vector.tensor_tensor(out=rowsum[0:NR2, :], in0=rowsum[0:NR2, :],
                                    in1=rs2[0:NR2, :], op=mybir.AluOpType.add)
            gsum = sb.tile([P, 1], F32, name="gsum")
            nc.gpsimd.partition_all_reduce(gsum[:], rowsum[:], P, ReduceOp.add)
            rinv = sb.tile([P, 1], F32, name="rinv")
            nc.vector.reciprocal(rinv[:], gsum[:])

            pi_sb = sb.tile([P, 2 * N], F32, name="pi_sb")
            nc.vector.tensor_scalar_mul(pi_sb[:, 0:N], esb[:, 0:N], rinv[:])
            nc.vector.tensor_scalar_mul(pi_sb[0:NR2, N:2 * N],
                                        esb[0:NR2, N:2 * N], rinv[0:NR2, :])

            nc.sync.dma_start(pi_d[0:P, :], pi_sb[0:P, 0:N])
            nc.sync.dma_start(pi_d[P:N, :], pi_sb[0:NR2, N:2 * N])

    nc.compile()
    return nc


_prog = None


def _get_program():
    global _prog
    if _prog is None:
        _prog = build_program()
    return _prog


def _build_wpack(i):
    w = np.zeros((P, WCOLS), np.float32)

    def put(col, arr):
        arr = np.asarray(arr, np.float32)
        if arr.ndim == 1:
            arr = arr[:, None]
        w[0:arr.shape[0], col:col + arr.shape[1]] = arr

    put(C_W1A0, i["g0_w1"])   # b1 cancels under BatchNorm
    put(C_W2A0, np.vstack([i["g0_w2"], i["g0_b2"][None, :]]))
    put(C_W1A1, i["g1_w1"])   # b1 cancels under BatchNorm
    put(C_W2A1, np.vstack([i["g1_w2"], i["g1_b2"][None, :]]))
    put(C_A1S, np.vstack([i["a_w1"][0:HID], i["a_b1"][None, :]]))
    put(C_A1P, i["a_w1"][HID:2 * HID])
    put(C_A1Q, i["a_w1"][2 * HID:3 * HID])
    w24 = np.zeros((P, 4), np.float32)
    for ii in range(4):
        w24[HA * ii:HA * (ii + 1), ii] = i["a_w2"][:, 0]
    put(C_W24, w24)
    put(C_C1, np.vstack([i["c_w1"], i["c_b1"][None, :]]))
    put(C_C2, np.vstack([i["c_w2"], i["c_b2"][None, :]]))
    put(C_G0, i["g0_gamma"])
    put(C_B0, i["g0_beta"])
    put(C_G1, i["g1_gamma"])
    put(C_B1, i["g1_beta"])
    return w.astype(NPBF16 if DENSE_BF16 else np.float32)


def kernel(**inputs):
    inp = {k: np.asarray(v) for k, v in inputs.items()}
    x = inp["x"].astype(np.float32)                      # [1800, 2]
    ei = inp["edge_index"].astype(np.int64)              # [2, 14400]

    adt = NPBF16 if ADJ_BF16 else np.float32
    at = np.zeros((NAP, NA), np.float32)
    np.add.at(at, (ei[0], ei[1]), 1.0)
    at = at.astype(adt)

    xp = np.zeros((NAP, 2), np.float32)
    xp[0:NA] = x
    xp = xp.astype(adt)
    wpack = _build_wpack(inp)

    in_maps = []
    for c in range(B):
        in_maps.append({
            "at_c": np.ascontiguousarray(at[:, c * N:(c + 1) * N]),
            "x_all": xp,
            "xt_c": np.ascontiguousarray(x[c * N:(c + 1) * N].T),
            "wpack": wpack,
        })

    nc = _get_program()
    res = run_bass_kernel_spmd(nc, in_maps, core_ids=list(range(B)))
    kernel._last_results = res

    pi = np.stack([res.results[c]["pi"].reshape(-1) for c in range(B)])
    val = np.concatenate([res.results[c]["val"].reshape(1, 1) for c in range(B)])
    return pi, val


# revision 44
# speedup vs baseline: 1.1090x; 1.0899x over previous
"""Trainium2 Bass kernel for nn_ActorCritic (GIN message passing actor-critic).

Strategy (data-parallel over graphs, per sharding hint):
- 8 graphs x 225 nodes; core c owns graph c (225 nodes).
- segment_sum over the 14400 random (cross-graph) edges is done as a dense
  bf16 matmul with a host-built adjacency shard AT_c[src, dst_local].
- GIN MLP runs feature-major ([feat, node]) so BatchNorm reductions are along
  the free dim. BN statistics are global over all 1800 nodes: each core
  computes its 225 pre-BN columns, an AllGather (bf16) shares them, every core
  redundantly computes the tiny global stats locally. rsqrt for BN runs on the
  vector engine (fast-inverse-sqrt + Newton) so ScalarE only ever loads the
  Relu and Exp activation tables.
- Actor head factorized: logits[i,j] = w2 . relu(s + Q[:,i] + P[:,j]);
  hidden tiles pack 4 i's x 32 k on 128 partitions (bf16); logits accumulate
  into one PSUM tile via shifted block-diagonal weights. Softmax is local.
"""

import sys

if "/opt/trn_rl_repo" not in sys.path:
    sys.path.insert(0, "/opt/trn_rl_repo")

import ml_dtypes
import numpy as np

import concourse.bacc as bacc
import concourse.bass as bass
import concourse.mybir as mybir
import concourse.tile as tile
from concourse.bass_isa import ReduceOp
from concourse.bass_utils import run_bass_kernel_spmd

P = 128
N = 225            # nodes per graph (= per core)
B = 8              # graphs = cores
NA = N * B         # 1800 total nodes
K = 15             # src tiles of 128
NAP = K * P        # 1920 padded
HID = 64
HA = 32            # actor hidden
HC = 32            # critic hidden
BN_EPS = 1e-5
F32 = mybir.dt.float32
BF16 = mybir.dt.bfloat16
NPBF16 = ml_dtypes.bfloat16

# wpack column layout ([128, WCOLS])
C_W1A0 = 0     # rows 0:2   g0_w1 (b1 cancels under BN)   cols 0:64
C_W2A0 = 64    # rows 0:65  [g0_w2; g0_b2]                cols 64:128
C_W1A1 = 128   # rows 0:64  g1_w1                          cols 128:192
C_W2A1 = 192   # rows 0:65  [g1_w2; g1_b2]                cols 192:256
C_A1S = 256    # rows 0:65  [a_w1[0:64]; a_b1]             cols 256:288
C_A1P = 288    # rows 0:64  a_w1[64:128]                   cols 288:320
C_A1Q = 320    # rows 0:64  a_w1[128:192]                  cols 320:352
C_W24 = 352    # rows 0:128 block-diag a_w2                cols 352:356
C_C1 = 356     # rows 0:65  [c_w1; c_b1]                   cols 356:388
C_C2 = 388     # rows 0:33  [c_w2; c_b2]                   cols 388:389
C_G0 = 389     # rows 0:64  g0_gamma
C_B0 = 390     # rows 0:64  g0_beta
C_G1 = 391     # rows 0:64  g1_gamma
C_B1 = 392     # rows 0:64  g1_beta
WCOLS = 400

import os
ADJ_BF16 = os.environ.get("KADJ", "1") == "1"     # adjacency matmul path
DENSE_BF16 = os.environ.get("KDENSE", "1") == "1" # dense-layer matmul path
ACTOR_BF16 = os.environ.get("KACTOR", "1") == "1" # actor hid/w24 matmuls
V1_STYLE = os.environ.get("KV1", "0") == "1"      # unused master switch
COPY_NEW = os.environ.get("KCOPY", "1") == "1"    # vector copy for h1fmcs
EMBA_NEW = os.environ.get("KEMBA", "1") == "1"    # reduce emb from PSUM featsp
SP4_NEW = os.environ.get("KSP4", "1") == "1"      # all-DVE sp4 build
SPLIT_NEW = os.environ.get("KSPLIT", "1") == "1"  # ACT_EVERY actor relu split
ADT = BF16 if ADJ_BF16 else F32
DDT = BF16 if DENSE_BF16 else F32
XDT = BF16 if ACTOR_BF16 else F32

NBLK = 32          # actor loop count (block b covers i-blocks b and b+32)
NIB = 57           # i blocks of 4 (225 valid i -> 57 blocks)
ACT_EVERY = 3      # 1 in ACT_EVERY actor relus go to ScalarE, rest to DVE


def _bn_scale_shift(nc, sb, pre_all, ncols, junk, gamma, beta, name):
    """Global BN (scale, shift) [HID,1] f32.

    pre_all is [HID, B, N+2] — per-rank blocks with local sum at col N and
    local sum-of-squares at col N+1 (computed pre-AllGather), so the global
    stats are tiny 8-element reductions instead of 1800-col passes."""
    s1 = sb.tile([HID, 1], F32, name=f"s1_{name}")
    s2 = sb.tile([HID, 1], F32, name=f"s2_{name}")
    nc.vector.reduce_sum(out=s1[:], in_=pre_all[:, :, N:N + 1],
                         axis=mybir.AxisListType.XY)
    nc.vector.reduce_sum(out=s2[:], in_=pre_all[:, :, N + 1:N + 2],
                         axis=mybir.AxisListType.XY)
    ncols = float(ncols)
    mean = sb.tile([HID, 1], F32, name=f"mean_{name}")
    var = sb.tile([HID, 1], F32, name=f"var_{name}")
    nc.vector.tensor_scalar_mul(mean[:], s1[:], 1.0 / ncols)
    nc.vector.tensor_scalar_mul(s2[:], s2[:], 1.0 / ncols)
    nc.vector.tensor_tensor(out=var[:], in0=mean[:], in1=mean[:], op=mybir.AluOpType.mult)
    nc.vector.tensor_tensor(out=var[:], in0=s2[:], in1=var[:], op=mybir.AluOpType.subtract)
    nc.vector.tensor_scalar_add(var[:], var[:], BN_EPS)
    nc.scalar.activation(out=var[:], in_=var[:], func=mybir.ActivationFunctionType.Sqrt)
    inv = sb.tile([HID, 1], F32, name=f"inv_{name}")
    nc.vector.reciprocal(inv[:], var[:])
    scale = sb.tile([HID, 1], F32, name=f"scale_{name}")
    shift = sb.tile([HID, 1], F32, name=f"shift_{name}")
    nc.vector.tensor_tensor(out=scale[:], in0=gamma, in1=inv[:], op=mybir.AluOpType.mult)
    nc.vector.tensor_tensor(out=inv[:], in0=mean[:], in1=scale[:], op=mybir.AluOpType.mult)
    nc.vector.tensor_tensor(out=shift[:], in0=beta, in1=inv[:], op=mybir.AluOpType.subtract)
    return scale, shift


def build_program():
    nc = bacc.Bacc("TRN2", target_bir_lowering=False, debug=False, num_devices=B)

    at_d = nc.dram_tensor("at_c", [NAP, N], ADT, kind="ExternalInput")
    x_d = nc.dram_tensor("x_all", [NAP, 2], ADT, kind="ExternalInput")
    xt_d = nc.dram_tensor("xt_c", [2, N], F32, kind="ExternalInput")
    wp_d = nc.dram_tensor("wpack", [P, WCOLS], DDT, kind="ExternalInput")
    pi_d = nc.dram_tensor("pi", [N, N], F32, kind="ExternalOutput")
    val_d = nc.dram_tensor("val", [1, 1], F32, kind="ExternalOutput")

    rg = [list(range(B))]
    RELU = mybir.ActivationFunctionType.Relu
    EXP = mybir.ActivationFunctionType.Exp

    with tile.TileContext(nc) as tc:
        with (
            tc.tile_pool(name="sb", bufs=1) as sb,
            tc.tile_pool(name="ps", bufs=4, space="PSUM") as ps,
            tc.tile_pool(name="psl", bufs=1, space="PSUM") as psl,
            tc.tile_pool(name="hidp", bufs=6) as hidp,
            tc.tile_pool(name="dram", bufs=1, space="DRAM") as dram,
        ):
            # Warm-up collective: absorbs the one-time collective-entry
            # setup (~25us observed on the first collective) during the
            # input-DMA phase instead of on the critical path.
            # Content is irrelevant (output never read), so no input DMA:
            # the collective has zero dependencies and issues immediately.
            warm_in = dram.tile([1, 16], F32, name="warm_in")
            warm_out = dram.tile([B, 16], F32, name="warm_out", addr_space="Shared")
            nc.gpsimd.collective_compute(
                "AllGather", mybir.AluOpType.bypass, replica_groups=rg,
                ins=[warm_in[:]], outs=[warm_out[:]],
            )

            # ---------------- loads ----------------
            at_sb = sb.tile([P, K, N], ADT, name="at_sb")
            at_v = at_d[:, :].rearrange("(k p) j -> p k j", p=P)
            for g, (k0, k1) in enumerate([(0, 2), (2, 4), (4, 6), (6, 8), (8, 10),
                                          (10, 12), (12, 14), (14, 15)]):
                eng = nc.sync if g % 2 == 0 else nc.scalar
                eng.dma_start(at_sb[:, k0:k1, :], at_v[:, k0:k1, :])
            x_sb = sb.tile([P, K, 2], ADT, name="x_sb")
            nc.sync.dma_start(x_sb[:], x_d[:, :].rearrange("(k p) c -> p k c", p=P))
            xt_sb = sb.tile([2, N], F32, name="xt_sb")
            nc.sync.dma_start(xt_sb[:], xt_d[:, :])
            wp = sb.tile([P, WCOLS], DDT, name="wp")
            nc.scalar.dma_start(wp[:], wp_d[:, :])

            gamma0 = wp[0:HID, C_G0:C_G0 + 1]
            beta0 = wp[0:HID, C_B0:C_B0 + 1]
            gamma1 = wp[0:HID, C_G1:C_G1 + 1]
            beta1 = wp[0:HID, C_B1:C_B1 + 1]

            # ---------------- GIN layer 0 ----------------
            agg0 = ps.tile([2, N], F32, name="agg0", tag="bank")
            for k in range(K):
                nc.tensor.matmul(agg0[:], x_sb[:, k, :], at_sb[:, k, :],
                                 start=(k == 0), stop=(k == K - 1))
            hin0 = sb.tile([2, N], DDT, name="hin0")
            nc.vector.tensor_tensor(out=hin0[:], in0=agg0[:], in1=xt_sb[:],
                                    op=mybir.AluOpType.add)
            pre0c = ps.tile([HID, N], F32, name="pre0c", tag="bank")
            nc.tensor.matmul(pre0c[:], wp[0:2, C_W1A0:C_W1A0 + HID], hin0[:],
                             start=True, stop=True)

            # AllGather pre-BN columns + local stats: payload [64, 227]
            NS = N + 2
            ag_in0 = dram.tile([HID, NS], BF16, name="ag_in0")
            ag_out0 = dram.tile([HID * B, NS], BF16, name="ag_out0", addr_space="Shared")
            pre0cs = sb.tile([HID, NS], BF16, name="pre0cs")
            junk = sb.tile([HID, N], F32, name="junk")
            sloc = sb.tile([HID, 2], F32, name="sloc")
            nc.vector.tensor_copy(pre0cs[:, 0:N], pre0c[:])
            nc.vector.reduce_sum(out=sloc[:, 0:1], in_=pre0c[:],
                                 axis=mybir.AxisListType.X)
            nc.scalar.activation(out=junk[:], in_=pre0c[:],
                                 func=mybir.ActivationFunctionType.Square,
                                 accum_out=sloc[:, 1:2])
            nc.vector.tensor_copy(pre0cs[:, N:N + 2], sloc[:])
            nc.sync.dma_start(ag_in0[:], pre0cs[:])
            nc.gpsimd.collective_compute(
                "AllGather", mybir.AluOpType.bypass, replica_groups=rg,
                ins=[ag_in0[:]], outs=[ag_out0[:]],
            )
            pre0all = sb.tile([HID, B, NS], BF16, name="pre0all")
            ag0v = ag_out0[:].rearrange("(r f) j -> f r j", f=HID)
            for r in range(0, B, 2):
                eng = nc.sync if (r // 2) % 2 == 0 else nc.scalar
                eng.dma_start(pre0all[:, r:r + 2, :], ag0v[:, r:r + 2, :])
            scale0, shift0 = _bn_scale_shift(nc, sb, pre0all, NA, junk,
                                             gamma0, beta0, "l0")

            # act0 (all nodes, feature-major bf16, +ones row); own-node copy
            act0 = sb.tile([HID + 1, NAP], DDT, name="act0")
            nc.vector.memset(act0[HID:HID + 1, :], 1.0)
            nc.vector.memset(act0[0:HID, NA:NAP], 0.0)
            nc.scalar.activation(
                out=act0[0:HID, 0:NA].rearrange("f (r j) -> f r j", r=B),
                in_=pre0all[:, :, 0:N],
                func=RELU, bias=shift0[:], scale=scale0[:])
            act0c = sb.tile([HID + 1, N], DDT, name="act0c")
            nc.vector.memset(act0c[HID:HID + 1, :], 1.0)
            nc.scalar.activation(out=act0c[0:HID, :], in_=pre0c[:],
                                 func=RELU, bias=shift0[:], scale=scale0[:])

            # h1 all nodes node-major (bf16) and own nodes feature-major
            h1nm = sb.tile([P, K, HID], ADT, name="h1nm")
            for k in range(K):
                hp = ps.tile([P, HID], F32, name=f"hp{k}", tag="bank")
                nc.tensor.matmul(hp[:], act0[:, k * P:(k + 1) * P],
                                 wp[0:HID + 1, C_W2A0:C_W2A0 + HID],
                                 start=True, stop=True)
                if k % 2 == 0:
                    nc.vector.tensor_copy(h1nm[:, k, :], hp[:])
                else:
                    nc.scalar.copy(h1nm[:, k, :], hp[:])
            h1fmc = ps.tile([HID, N], F32, name="h1fmc", tag="bank")
            nc.tensor.matmul(h1fmc[:], wp[0:HID + 1, C_W2A0:C_W2A0 + HID], act0c[:],
                             start=True, stop=True)
            h1fmcs = sb.tile([HID, N], F32, name="h1fmcs")
            if not COPY_NEW:
                nc.scalar.copy(h1fmcs[:], h1fmc[:])
            else:
                nc.vector.tensor_copy(h1fmcs[:], h1fmc[:])

            # ---------------- GIN layer 1 ----------------
            agg1 = ps.tile([HID, N], F32, name="agg1", tag="bank")
            for k in range(K):
                nc.tensor.matmul(agg1[:], h1nm[:, k, :], at_sb[:, k, :],
                                 start=(k == 0), stop=(k == K - 1))
            hin1 = sb.tile([HID, N], DDT, name="hin1")
            nc.vector.tensor_tensor(out=hin1[:], in0=agg1[:], in1=h1fmcs[:],
                                    op=mybir.AluOpType.add)
            pre1c = ps.tile([HID, N], F32, name="pre1c", tag="bank")
            nc.tensor.matmul(pre1c[:], wp[0:HID, C_W1A1:C_W1A1 + HID], hin1[:],
                             start=True, stop=True)

            ag_in1 = dram.tile([HID, NS], BF16, name="ag_in1")
            ag_out1 = dram.tile([HID * B, NS], BF16, name="ag_out1", addr_space="Shared")
            pre1cs = sb.tile([HID, NS], BF16, name="pre1cs")
            nc.vector.tensor_copy(pre1cs[:, 0:N], pre1c[:])
            nc.vector.reduce_sum(out=sloc[:, 0:1], in_=pre1c[:],
                                 axis=mybir.AxisListType.X)
            nc.scalar.activation(out=junk[:], in_=pre1c[:],
                                 func=mybir.ActivationFunctionType.Square,
                                 accum_out=sloc[:, 1:2])
            nc.vector.tensor_copy(pre1cs[:, N:N + 2], sloc[:])
            nc.sync.dma_start(ag_in1[:], pre1cs[:])
            nc.gpsimd.collective_compute(
                "AllGather", mybir.AluOpType.bypass, replica_groups=rg,
                ins=[ag_in1[:]], outs=[ag_out1[:]],
            )
            pre1all = sb.tile([HID, B, NS], BF16, name="pre1all")
            ag1v = ag_out1[:].rearrange("(r f) j -> f r j", f=HID)
            for r in range(0, B, 2):
                eng = nc.sync if (r // 2) % 2 == 0 else nc.scalar
                eng.dma_start(pre1all[:, r:r + 2, :], ag1v[:, r:r + 2, :])
            scale1, shift1 = _bn_scale_shift(nc, sb, pre1all, NA, junk,
                                             gamma1, beta1, "l1")

            act1c = sb.tile([HID + 1, N], DDT, name="act1c")
            nc.vector.memset(act1c[HID:HID + 1, :], 1.0)
            nc.scalar.activation(out=act1c[0:HID, :], in_=pre1c[:],
                                 func=RELU, bias=shift1[:], scale=scale1[:])

            featsp = ps.tile([HID, N], F32, name="featsp", tag="bank")
            nc.tensor.matmul(featsp[:], wp[0:HID + 1, C_W2A1:C_W2A1 + HID], act1c[:],
                             start=True, stop=True)
            feats = sb.tile([HID, N], DDT, name="feats")
            nc.vector.tensor_copy(feats[:], featsp[:])

            # ---------------- pooling + critic ----------------
            emba = sb.tile([HID + 1, 1], DDT, name="emba")
            nc.vector.memset(emba[HID:HID + 1, :], 1.0)
            embr = sb.tile([HID, 1], F32, name="embr")
            nc.vector.reduce_sum(out=embr[:], in_=featsp[:] if EMBA_NEW else feats[:],
                                 axis=mybir.AxisListType.X)
            nc.vector.tensor_scalar_mul(emba[0:HID, :], embr[:], 1.0 / N)

            v1p = ps.tile([HC, 1], F32, name="v1p", tag="bank")
            nc.tensor.matmul(v1p[:], wp[0:HID + 1, C_C1:C_C1 + HC], emba[:],
                             start=True, stop=True)
            v1a = sb.tile([HC + 1, 1], DDT, name="v1a")
            nc.vector.memset(v1a[HC:HC + 1, :], 1.0)
            nc.scalar.activation(out=v1a[0:HC, :], in_=v1p[:], func=RELU)
            valp = ps.tile([1, 1], F32, name="valp", tag="bank")
            nc.tensor.matmul(valp[:], wp[0:HC + 1, C_C2:C_C2 + 1], v1a[:],
                             start=True, stop=True)
            vals = sb.tile([1, 1], F32, name="vals")
            nc.vector.tensor_copy(vals[:], valp[:])
            nc.sync.dma_start(val_d[:, :], vals[:])

            # ---------------- actor head ----------------
            pp = ps.tile([HA, N], F32, name="pp", tag="bank")
            nc.tensor.matmul(pp[:], wp[0:HID, C_A1P:C_A1P + HA], feats[:],
                             start=True, stop=True)
            qp = ps.tile([HA, N], F32, name="qp", tag="bank")
            nc.tensor.matmul(qp[:], wp[0:HID, C_A1Q:C_A1Q + HA], feats[:],
                             start=True, stop=True)
            spp = ps.tile([HA, 1], F32, name="spp", tag="bank")
            nc.tensor.matmul(spp[:], wp[0:HID + 1, C_A1S:C_A1S + HA], emba[:],
                             start=True, stop=True)
            ssb = sb.tile([HA, 1], F32, name="ssb")
            nc.vector.tensor_copy(ssb[:], spp[:])

            # SP4 = 4x partition-stacked (P + s)
            sp4 = sb.tile([P, N], XDT, name="sp4")
            for ii in range(4):
                if (not SP4_NEW) and ii % 2 == 1:
                    nc.scalar.activation(out=sp4[HA * ii:HA * (ii + 1), :], in_=pp[:],
                                         func=mybir.ActivationFunctionType.Identity,
                                         bias=ssb[:])
                else:
                    nc.vector.tensor_scalar_add(sp4[HA * ii:HA * (ii + 1), :],
                                                pp[:], ssb[:])

            # Q4[32*ii + k, b] = Q[k, 4b + ii]
            qsb = sb.tile([HA, 228], F32, name="qsb")
            nc.vector.memset(qsb[:, N:228], 0.0)
            nc.vector.tensor_copy(qsb[:, 0:N], qp[:])
            q4 = sb.tile([P, 2 * NBLK], F32, name="q4")
            nc.vector.memset(q4[:], 0.0)
            qsb_r = qsb[:].rearrange("k (b i) -> k b i", i=4)
            for ii in range(4):
                nc.vector.tensor_copy(q4[HA * ii:HA * (ii + 1), 0:NIB],
                                      qsb_r[:, :, ii])

            # block-diagonal w2 stacks: lhsT_b is [128, 32], placed at column
            # 4*(b%8)+ii; matmul b accumulates into logits[32g:32g+32] with
            # tile_position=(0, 32g), g=b//8 — four independent PE col-groups
            # run concurrently.
            w24dev = sb.tile([P, NBLK, HA], XDT, name="w24dev")
            nc.gpsimd.memset(w24dev[:], 0.0)
            for b in range(NBLK):
                off = 4 * (b % 8)
                nc.vector.tensor_copy(w24dev[:, b, off:off + 4],
                                      wp[0:P, C_W24:C_W24 + 4])

            def relu_hid(eng_i, out_ap, bias_ap):
                if eng_i % 4 == 0:
                    nc.scalar.activation(out=out_ap, in_=sp4[:], func=RELU,
                                         bias=bias_ap)
                else:
                    nc.vector.tensor_scalar(
                        out=out_ap, in0=sp4[:], scalar1=bias_ap, scalar2=0.0,
                        op0=mybir.AluOpType.add, op1=mybir.AluOpType.max)

            # free size padded to 512 so each partition slice is bank-aligned
            logits_t = psl.tile([P, 512], F32, name="logits_t")
            # interleave groups so consecutive matmuls hit different col-groups
            order = [8 * (i % 4) + (i // 4) for i in range(NBLK)]
            for n_i, b in enumerate(order):
                g = b // 8
                # [128, 512] with region 2 at col 256: both regions start
                # 512B-aligned so the DVE 4x bf16 mode engages (143ns vs
                # 445ns measured for the unaligned 225-offset variant).
                hid = hidp.tile([P, 512], XDT, name="hid", tag="hid")
                relu_hid(2 * n_i, hid[:, 0:N], q4[:, b:b + 1])
                relu_hid(2 * n_i + 1, hid[:, 256:256 + N],
                         q4[:, NBLK + b:NBLK + b + 1])
                rhs = hid[:].rearrange("p (g c) -> p g c", c=256)[:, :, 0:N]
                nc.tensor.matmul(logits_t[32 * g:32 * g + 32, 0:2 * N],
                                 w24dev[:, b, :], rhs,
                                 start=(b % 8 == 0), stop=(b % 8 == 7),
                                 tile_position=(0, 32 * g),
                                 skip_group_check=True)

            # ---------------- softmax over all 50625 ----------------
            # Region 1 (cols 0:225): i=0..127 on all 128 partitions; region 2
            # (cols 225:450): i=128..224 on partitions 0:97. Partitions 97:128
            # of region 2 are garbage (i>=225): never read.
            NR2 = N - P
            rowmax = sb.tile([P, 1], F32, name="rowmax")
            rm2 = sb.tile([P, 1], F32, name="rm2")
            nc.vector.reduce_max(out=rowmax[:], in_=logits_t[:, 0:N],
                                 axis=mybir.AxisListType.X)
            nc.vector.reduce_max(out=rm2[0:NR2, :], in_=logits_t[0:NR2, N:2 * N],
                                 axis=mybir.AxisListType.X)
            nc.vector.tensor_tensor(out=rowmax[0:NR2, :], in0=rowmax[0:NR2, :],
                                    in1=rm2[0:NR2, :], op=mybir.AluOpType.max)
            gmax = sb.tile([P, 1], F32, name="gmax")
            nc.gpsimd.partition_all_reduce(gmax[:], rowmax[:], P, ReduceOp.max)
            nc.vector.tensor_scalar_mul(gmax[:], gmax[:], -1.0)

            esb = sb.tile([P, 2 * N], F32, name="esb")
            rowsum = sb.tile([P, 1], F32, name="rowsum")
            rs2 = sb.tile([P, 1], F32, name="rs2")
            nc.scalar.activation(out=esb[:, 0:N], in_=logits_t[:, 0:N],
                                 func=EXP, bias=gmax[:], accum_out=rowsum[:])
            nc.scalar.activation(out=esb[0:NR2, N:2 * N], in_=logits_t[0:NR2, N:2 * N],
                                 func=EXP, bias=gmax[0:NR2, :],
                                 accum_out=rs2[0:NR2, :])
            nc.vector.tensor_tensor(out=rowsum[0:NR2, :], in0=rowsum[0:NR2, :],
                                    in1=rs2[0:NR2, :], op=mybir.AluOpType.add)
            gsum = sb.tile([P, 1], F32, name="gsum")
            nc.gpsimd.partition_all_reduce(gsum[:], rowsum[:], P, ReduceOp.add)
            rinv = sb.tile([P, 1], F32, name="rinv")
            nc.vector.reciprocal(rinv[:], gsum[:])

            # scale + store in 4 partition chunks across both HWDGE engines
            # so the output DMAs pipeline with the scaling instead of
            # draining serially at the very end.
            pi_sb = sb.tile([P, 2 * N], F32, name="pi_sb")
            nc.vector.tensor_scalar_mul(pi_sb[0:64, 0:N], esb[0:64, 0:N],
                                        rinv[0:64, :])
            nc.sync.dma_start(pi_d[0:64, :], pi_sb[0:64, 0:N])
            nc.vector.tensor_scalar_mul(pi_sb[64:P, 0:N], esb[64:P, 0:N],
                                        rinv[64:P, :])
            nc.scalar.dma_start(pi_d[64:P, :], pi_sb[64:P, 0:N])
            nc.vector.tensor_scalar_mul(pi_sb[0:64, N:2 * N], esb[0:64, N:2 * N],
                                        rinv[0:64, :])
            nc.sync.dma_start(pi_d[P:P + 64, :], pi_sb[0:64, N:2 * N])
            nc.vector.tensor_scalar_mul(pi_sb[64:NR2, N:2 * N],
                                        esb[64:NR2, N:2 * N], rinv[64:NR2, :])
            nc.scalar.dma_start(pi_d[P + 64:N, :], pi_sb[64:NR2, N:2 * N])

    nc.compile()
    return nc


_prog = None


def _get_program():
    global _prog
    if _prog is None:
        _prog = build_program()
    return _prog


def _build_wpack(i):
    w = np.zeros((P, WCOLS), np.float32)

    def put(col, arr):
        arr = np.asarray(arr, np.float32)
        if arr.ndim == 1:
            arr = arr[:, None]
        w[0:arr.shape[0], col:col + arr.shape[1]] = arr

    put(C_W1A0, i["g0_w1"])   # b1 cancels under BatchNorm
    put(C_W2A0, np.vstack([i["g0_w2"], i["g0_b2"][None, :]]))
    put(C_W1A1, i["g1_w1"])   # b1 cancels under BatchNorm
    put(C_W2A1, np.vstack([i["g1_w2"], i["g1_b2"][None, :]]))
    put(C_A1S, np.vstack([i["a_w1"][0:HID], i["a_b1"][None, :]]))
    put(C_A1P, i["a_w1"][HID:2 * HID])
    put(C_A1Q, i["a_w1"][2 * HID:3 * HID])
    w24 = np.zeros((P, 4), np.float32)
    for ii in range(4):
        w24[HA * ii:HA * (ii + 1), ii] = i["a_w2"][:, 0]
    put(C_W24, w24)
    put(C_C1, np.vstack([i["c_w1"], i["c_b1"][None, :]]))
    put(C_C2, np.vstack([i["c_w2"], i["c_b2"][None, :]]))
    put(C_G0, i["g0_gamma"])
    put(C_B0, i["g0_beta"])
    put(C_G1, i["g1_gamma"])
    put(C_B1, i["g1_beta"])
    return w.astype(NPBF16 if DENSE_BF16 else np.float32)


def kernel(**inputs):
    inp = {k: np.asarray(v) for k, v in inputs.items()}
    x = inp["x"].astype(np.float32)                      # [1800, 2]
    ei = inp["edge_index"].astype(np.int64)              # [2, 14400]

    adt = NPBF16 if ADJ_BF16 else np.float32
    at = np.zeros((NAP, NA), np.float32)
    np.add.at(at, (ei[0], ei[1]), 1.0)
    at = at.astype(adt)

    xp = np.zeros((NAP, 2), np.float32)
    xp[0:NA] = x
    xp = xp.astype(adt)
    wpack = _build_wpack(inp)

    in_maps = []
    for c in range(B):
        in_maps.append({
            "at_c": np.ascontiguousarray(at[:, c * N:(c + 1) * N]),
            "x_all": xp,
            "xt_c": np.ascontiguousarray(x[c * N:(c + 1) * N].T),
            "wpack": wpack,
        })

    nc = _get_program()
    res = run_bass_kernel_spmd(nc, in_maps, core_ids=list(range(B)))
    kernel._last_results = res

    pi = np.stack([res.results[c]["pi"].reshape(-1) for c in range(B)])
    val = np.concatenate([res.results[c]["val"].reshape(1, 1) for c in range(B)])
    return pi, val
